# revision 46
# baseline (speedup 1.0000x reference)
# Trainium2 Bass kernel for DistNSA (sparse attention, 3 branches).
#
# Strategy (v2): causal-balanced query sharding with per-core PACKED keys.
#   - Core c owns two 128-query tiles: shallow t0=c and deep t1=8+c.  Every
#     core's key space is re-packed by the host into 16 tiles covering global
#     tiles (c-7..c+8) (zero-padded on the left), so the shallow slot's causal
#     span is always packed tiles 0..7 and the deep slot's is 0..15 with the
#     diagonal at the end.  This keeps the SPMD program uniform while cutting
#     the dense-attention work from 32 to 24 key-tile units per core.
#   - Everything is computed in the transposed E-layout [keys(part),
#     (head, query)(free)], 4 heads of a kv group batched per matmul.
#   - Window branch: only the last 5 packed tiles can intersect the 512-key
#     window; middle tiles are used unmasked (mask==1 there), the band and
#     diagonal (and shallow padding) get host masks.
#   - Selected branch: block-selection one-hots are expanded to keys with a
#     small matmul and multiplied into exp(logits); padded/invalid blocks are
#     never selected so padded tiles contribute exactly zero.
#   - Normalization + gating + branch combine happen on the HOST: the kernel
#     DMAs out unnormalized PV accumulators and Z rows per branch, plus the
#     compressed-branch output and its softmax denominators.
#   - Selection scores (compressed branch) stay in fp32 end-to-end (host-
#     computed fp32 block means, fp32 PE matmul, fp32 softmax) to reproduce
#     jax's top-k tie-breaking exactly; the heavy attention path runs fp16.
import numpy as np

import concourse.bass as bass
import concourse.bacc as bacc_mod
import concourse.mybir as mybir
from concourse.tile import TileContext

F32 = mybir.dt.float32
F32R = mybir.dt.float32r
E16 = mybir.dt.float16
U32 = mybir.dt.uint32
AOT = mybir.ActivationFunctionType
ALU = mybir.AluOpType

S, NHQ, NHK, HD = 2048, 8, 2, 128
REP = NHQ // NHK
WIN, BLK, NB, TOPN = 512, 32, 64, 4
SCALE = float(HD) ** -0.5
NCORE = 8
QT = 128                 # queries per slot
L0, L1 = 8, 16           # packed causal spans (key tiles) per slot
NEG_EPS = 1e-30


def build_nc() -> bass.Bass:
    nc = bacc_mod.Bacc("TRN2", target_bir_lowering=False, debug=False)

    # ---------------- DRAM I/O ----------------
    qTp_d = nc.dram_tensor("qTp", [128, 2, NHQ, QT], E16, kind="ExternalInput")
    qT32_d = nc.dram_tensor("qT32", [128, 2, NHQ, QT], F32, kind="ExternalInput")
    kTp_d = nc.dram_tensor("kTp", [128, NHK, S], E16, kind="ExternalInput")
    vbp_d = nc.dram_tensor("vbp", [128, NHK, 16, HD], E16, kind="ExternalInput")
    kcT_d = nc.dram_tensor("kcT", [128, NHK, NB], F32, kind="ExternalInput")
    vcs_d = nc.dram_tensor("vcs", [NB, NHK, HD], E16, kind="ExternalInput")
    caus_d = nc.dram_tensor("caus4", [128, REP, 128], E16, kind="ExternalInput")
    winm_d = nc.dram_tensor("winm4", [128, 4, REP, 128], E16, kind="ExternalInput")
    winb_d = nc.dram_tensor("winb4", [128, REP, 128], E16, kind="ExternalInput")
    nval_d = nc.dram_tensor("nvalid", [128, 2], F32, kind="ExternalInput")
    bon_d = nc.dram_tensor("bonus", [128, 2, NB], F32, kind="ExternalInput")
    io64_d = nc.dram_tensor("iota64", [128, NB], F32, kind="ExternalInput")
    ex01_d = nc.dram_tensor("expand01", [NB, S], E16, kind="ExternalInput")
    idb_d = nc.dram_tensor("identh", [128, 128], E16, kind="ExternalInput")
    on128_d = nc.dram_tensor("ones128", [128, 128], E16, kind="ExternalInput")

    opvw_d = nc.dram_tensor("opvw", [2, NHK, 128, 512], E16, kind="ExternalOutput")
    opvs_d = nc.dram_tensor("opvs", [2, NHK, 128, 512], E16, kind="ExternalOutput")
    zr_d = nc.dram_tensor("zrows", [2, NHK, 2, 512], F32, kind="ExternalOutput")
    ocp_d = nc.dram_tensor("ocp", [2, NHK, 128, 512], E16, kind="ExternalOutput")
    rc_d = nc.dram_tensor("rc", [128, 2, NHQ], F32, kind="ExternalOutput")

    from contextlib import ExitStack

    with TileContext(nc) as tc, ExitStack() as ctx:
        cpool = ctx.enter_context(tc.tile_pool(name="const", bufs=1))
        apool = ctx.enter_context(tc.tile_pool(name="aphase", bufs=2))
        epool = ctx.enter_context(tc.tile_pool(name="espace", bufs=2))
        psQK = ctx.enter_context(tc.tile_pool(name="psQK", bufs=2, space="PSUM"))
        psSel = ctx.enter_context(tc.tile_pool(name="psSel", bufs=2, space="PSUM"))
        psPV = ctx.enter_context(tc.tile_pool(name="psPV", bufs=1, space="PSUM"))

        # ------- persistent loads, ordered by first consumption -------
        nval = cpool.tile([128, 2], F32, name="nval_s")
        nc.sync.dma_start(nval, nval_d[:])
        io64 = cpool.tile([128, NB], F32, name="io64_s")
        nc.sync.dma_start(io64, io64_d[:])
        kcT = cpool.tile([128, NHK, NB], F32, name="kcT_s")
        nc.sync.dma_start(kcT, kcT_d[:])
        bon = cpool.tile([128, 2, NB], F32, name="bon_s")
        nc.sync.dma_start(bon, bon_d[:])
        idb = cpool.tile([128, 128], E16, name="idb_s")
        nc.sync.dma_start(idb, idb_d[:])
        qT32 = cpool.tile([128, 2, NHQ, QT], F32, name="qT32_s")
        nc.sync.dma_start(qT32[:, 1], qT32_d[:, 1])
        vcs = cpool.tile([NB, NHK, HD], E16, name="vcs_s")
        nc.sync.dma_start(vcs, vcs_d[:])
        qTp = cpool.tile([128, 2, NHQ, QT], E16, name="qTp_s")
        nc.sync.dma_start(qTp, qTp_d[:])
        kTp = cpool.tile([128, NHK, S], E16, name="kTp_s")
        nc.sync.dma_start(kTp[:, 0], kTp_d[:, 0])
        nc.sync.dma_start(qT32[:, 0], qT32_d[:, 0])
        ex01 = cpool.tile([NB, S], E16, name="ex01_s")
        nc.sync.dma_start(ex01, ex01_d[:])
        winb4 = cpool.tile([128, REP, 128], E16, name="winb4_s")
        nc.sync.dma_start(winb4, winb_d[:])
        caus4 = cpool.tile([128, REP, 128], E16, name="caus4_s")
        nc.sync.dma_start(caus4, caus_d[:])
        on128 = cpool.tile([128, 128], E16, name="on128_s")
        nc.sync.dma_start(on128, on128_d[:])
        vbp = cpool.tile([128, NHK, 16, HD], E16, name="vbp_s")
        nc.sync.dma_start(vbp[:, 0], vbp_d[:, 0])
        nc.sync.dma_start(kTp[:, 1], kTp_d[:, 1])
        nc.sync.dma_start(vbp[:, 1], vbp_d[:, 1])
        winm4 = cpool.tile([128, 4, REP, 128], E16, name="winm4_s")
        nc.sync.dma_start(winm4, winm_d[:])

        bTs4 = cpool.tile([NB, 2, NHK, QT], E16, name="bTs4_s")
        rc8 = cpool.tile([128, 2, NHQ], F32, name="rc8_s")

        # ---------------- phase A: cmp branch + selection ----------------
        def phase_a_gen(j):
            negc8 = apool.tile([128, NHQ, NB], F32, name="negc8", tag="negc8")
            nc.vector.tensor_scalar(
                negc8, io64[:, None, :].broadcast_to([128, NHQ, NB]),
                nval[:, j:j + 1], -1e30, op0=ALU.is_ge, op1=ALU.mult,
            )
            lc8 = psQK.tile([128, NHQ, NB], F32, name="lc8", tag="qk")
            for g in range(NHK):
                for r in range(REP):
                    h = g * REP + r
                    nc.tensor.matmul(lc8[:, h], qT32[:, j, h], kcT[:, g])
            yield
            lcm8 = apool.tile([128, NHQ, NB], F32, name="lcm8", tag="lcm8")
            nc.vector.scalar_tensor_tensor(
                out=lcm8, in0=lc8, scalar=SCALE,
                in1=negc8,
                op0=ALU.mult, op1=ALU.add,
            )
            ec8 = apool.tile([128, NHQ, NB], F32, name="ec8", tag="ec8")
            nc.scalar.activation(ec8, lcm8, AOT.Exp)
            zc8 = apool.tile([128, NHQ], F32, name="zc8", tag="zc8")
            nc.vector.tensor_reduce(
                out=zc8, in_=ec8, axis=mybir.AxisListType.X, op=ALU.add,
            )
            nc.vector.tensor_scalar_add(zc8, zc8, NEG_EPS)
            nc.vector.reciprocal(rc8[:, j], zc8)
            ecb8 = apool.tile([128, NHQ, NB], E16, name="ecb8", tag="ecb8")
            nc.vector.tensor_copy(ecb8, ec8)
            yield

            for g in range(NHK):
                pg = [
                    apool.tile([128, NB], F32, name=f"pg{i}", tag=f"pg{i}")
                    for i in range(2)
                ]
                for r in range(REP):
                    h = g * REP + r
                    if r == 0:
                        nc.vector.tensor_scalar(
                            pg[0], ec8[:, h], rc8[:, j, h:h + 1], None,
                            op0=ALU.mult,
                        )
                    else:
                        nc.vector.scalar_tensor_tensor(
                            out=pg[r % 2], in0=ec8[:, h],
                            scalar=rc8[:, j, h:h + 1],
                            in1=pg[(r + 1) % 2], op0=ALU.mult, op1=ALU.add,
                        )
                score = apool.tile([128, NB], F32, name="score", tag="score")
                nc.vector.tensor_add(score, pg[(REP - 1) % 2], bon[:, j])
                mx8 = apool.tile([128, 8], F32, name="mx8", tag="mx8")
                nc.vector.max(out=mx8, in_=score)
                ix8 = apool.tile([128, 8], U32, name="ix8", tag="ix8")
                nc.vector.max_index(ix8, mx8, score)
                ixf = apool.tile([128, TOPN], F32, name="ixf", tag="ixf")
                nc.vector.tensor_copy(ixf, ix8[:, :TOPN])
                bsel = [
                    apool.tile([128, NB], E16, name=f"bsel{i}", tag=f"bsel{i}")
                    for i in range(2)
                ]
                nc.vector.tensor_scalar(
                    bsel[0], io64, ixf[:, 0:1], None, op0=ALU.is_equal
                )
                for t in range(1, TOPN):
                    nc.vector.scalar_tensor_tensor(
                        out=bsel[t % 2], in0=io64, scalar=ixf[:, t:t + 1],
                        in1=bsel[(t + 1) % 2], op0=ALU.is_equal, op1=ALU.add,
                    )
                btp = psQK.tile([NB, 128], E16, name="btp", tag="qk")
                nc.tensor.transpose(btp, bsel[(TOPN - 1) % 2], idb)
                nc.vector.tensor_copy(bTs4[:, j, g], btp)
                # cmp PV: transpose ec per head, PV against block-mean V
                yield
                ocp4 = psQK.tile([128, REP, HD], F32, name="ocp4", tag="qk")
                for r in range(REP):
                    h = g * REP + r
                    ectp = psSel.tile([NB, 128], E16, name="ectp", tag="sel")
                    nc.tensor.transpose(ectp, ecb8[:, h], idb)
                    ecT = apool.tile([NB, 128], E16, name="ecT", tag="ecT")
                    if r % 2:
                        nc.scalar.activation(ecT, ectp, AOT.Copy)
                    else:
                        nc.vector.tensor_copy(ecT, ectp)
                    nc.tensor.matmul(ocp4[:, r], ecT, vcs[:, g])
                oc16 = apool.tile([128, REP, HD], E16, name="oc16", tag="oc16")
                nc.scalar.activation(oc16, ocp4, AOT.Copy)
                nc.sync.dma_start(ocp_d[j, g], oc16)
                yield

        # --------- phase B emission helpers (kt-pair granular) ---------
        def b_alloc(j, g):
            espb = epool.tile([128, 16, REP, 128], E16, name="espb", tag="espb")
            esb = epool.tile([128, 16, REP, 128], E16, name="esb", tag="esb")
            return {"j": j, "g": g, "L": L1 if j else L0,
                    "espb": espb, "esb": esb, "eww": {}, "nmul": 0}

        FILL_TAGS = ("qk", "opvw", "opvs", "zw", "zs", "sel")

        def b_pair_qk(st, pk, fill=False):
            # QK + exp for kts (2pk, 2pk+1), single-kt psum tiles so the PE
            # can run a tile ahead of the Scalar exp.  During the fill the PV
            # banks are still idle, so borrow them for a 5-deep ring.
            j, g, espb = st["j"], st["g"], st["espb"]
            for i in range(2):
                kt = 2 * pk + i
                if fill:
                    tag = FILL_TAGS[kt % len(FILL_TAGS)]
                    pool = (psQK if tag == "qk"
                            else psSel if tag == "sel" else psPV)
                else:
                    tag, pool = "qk", psQK
                qk = pool.tile([128, REP, 128], F32, name="qk", tag=tag)
                nc.tensor.matmul(
                    qk, kTp[:, g, kt * 128:(kt + 1) * 128],
                    qTp[:, j, g * REP:(g + 1) * REP],
                )
                nc.scalar.activation(espb[:, kt], qk, AOT.Exp, scale=SCALE)

        def b_pair_sel(st, pk):
            # selection expand + es-mul for kt pair
            j, g, L, espb, esb = st["j"], st["g"], st["L"], st["espb"], st["esb"]
            kt0 = 2 * pk
            for i in range(2):
                kt = kt0 + i
                sel = psSel.tile([128, 128], F32, name="sel", tag="sel")
                nc.tensor.matmul(
                    sel, ex01[:, kt * 128:(kt + 1) * 128], bTs4[:, j, g]
                )
                if kt == L - 1:
                    st["sel_diag"] = sel[:, None, :].broadcast_to([128, REP, 128])
                else:
                    st["nmul"] += 1
                    m = st["nmul"] % 3
                    if m == 0:
                        nc.vector.tensor_mul(
                            esb[:, kt], espb[:, kt],
                            sel[:, None, :].broadcast_to([128, REP, 128]),
                        )
                    else:
                        s16 = epool.tile([128, 128], E16, name="s16", tag="s16")
                        nc.scalar.activation(s16, sel, AOT.Copy)
                        eng = nc.vector if m == 1 else nc.gpsimd
                        eng.tensor_mul(
                            esb[:, kt], espb[:, kt],
                            s16[:, None, :].broadcast_to([128, REP, 128]),
                        )
                # window-branch masked tiles, emitted as soon as espb exists
                if j == 0 and 3 <= kt <= 6:
                    if "ewm" not in st:
                        st["ewm"] = epool.tile(
                            [128, 4, REP, 128], E16, name="ewm", tag="ewm")
                    i = kt - 3
                    eng = nc.gpsimd if i % 2 else nc.vector
                    eng.tensor_mul(st["ewm"][:, i], espb[:, kt], winm4[:, i])
                    st["eww"][kt] = st["ewm"][:, i]
                elif j == 1 and kt == L1 - 5:
                    ewb = epool.tile([128, REP, 128], E16, name="ewb", tag="ewb")
                    nc.vector.tensor_mul(ewb, espb[:, kt], winb4)
                    st["eww"][kt] = ewb

        def b_tail(st):
            # diagonal tile: causal-masked window + selected
            L, espb, esb = st["L"], st["espb"], st["esb"]
            ewd = epool.tile([128, REP, 128], E16, name="ewd", tag="ewd")
            nc.vector.tensor_mul(ewd, espb[:, L - 1], caus4)
            st["eww"][L - 1] = ewd
            nc.vector.tensor_mul(esb[:, L - 1], ewd, st["sel_diag"])

        # --------- phase C emission helpers ---------
        def c_alloc(st):
            st["opvw"] = psPV.tile([128, 512], F32, name="opvw", tag="opvw")
            st["opvs"] = psPV.tile([128, 512], F32, name="opvs", tag="opvs")
            st["zw"] = psPV.tile([128, 512], F32, name="zw", tag="zw")
            st["zs"] = psPV.tile([128, 512], F32, name="zs", tag="zs")

        def c_kt(st, kt):
            j, g, L = st["j"], st["g"], st["L"]
            w0 = L - 5
            sst, ssp = (kt == 0), (kt == L - 1)
            nc.tensor.matmul(st["opvs"], vbp[:, g, kt], st["esb"][:, kt],
                             start=sst, stop=ssp)
            nc.tensor.matmul(st["zs"], on128, st["esb"][:, kt],
                             start=sst, stop=ssp)
            if kt >= w0:
                rhs_w = st["eww"].get(kt)
                if rhs_w is None:
                    rhs_w = st["espb"][:, kt]
                nc.tensor.matmul(st["opvw"], vbp[:, g, kt], rhs_w,
                                 start=(kt == w0), stop=ssp)
                nc.tensor.matmul(st["zw"], on128, rhs_w,
                                 start=(kt == w0), stop=ssp)

        def c_out(st, split=1):
            j, g = st["j"], st["g"]
            ow16 = epool.tile([128, 512], E16, name="ow16", tag="ow16")
            nc.scalar.activation(ow16, st["opvw"], AOT.Copy)
            nc.sync.dma_start(opvw_d[j, g], ow16)
            os16 = epool.tile([128, 512], E16, name="os16", tag="os16")
            nc.vector.tensor_copy(os16, st["opvs"])
            nc.sync.dma_start(opvs_d[j, g], os16)
            zrw = epool.tile([1, 512], F32, name="zrw", tag="zrw")
            nc.scalar.activation(zrw, st["zw"][0:1, :], AOT.Copy)
            nc.sync.dma_start(zr_d[j, g, 0:1], zrw)
            zrs = epool.tile([1, 512], F32, name="zrs", tag="zrs")
            nc.vector.tensor_copy(zrs, st["zs"][0:1, :])
            nc.sync.dma_start(zr_d[j, g, 1:2], zrs)

        def stage(nxt, cur, cb=None):
            # 2-deep software pipeline step: the NEXT unit's QK/exp pairs
            # (Scalar-paced) stream while the CURRENT unit's selection
            # expansion, es-muls and PV/Z accumulation run.
            if cur is not None:
                c_alloc(cur)
            npair_n = (nxt["L"] // 2) if nxt is not None else 0
            npair_c = (cur["L"] // 2) if cur is not None else 0
            for pk in range(max(npair_n, npair_c)):
                if pk < npair_n:
                    b_pair_qk(nxt, pk)
                if pk < npair_c:
                    b_pair_sel(cur, pk)
                    if pk >= 2:
                        c_kt(cur, 2 * pk - 4)
                        c_kt(cur, 2 * pk - 3)
                if cb:
                    cb()
            if cur is not None:
                b_tail(cur)
                for kt in range(max(0, cur["L"] - 4), cur["L"]):
                    c_kt(cur, kt)
                c_out(cur, split=4 if nxt is None else 1)

        # ---------------- emission schedule ----------------
        # PE warm-up during the input-DMA window: the HAM clock gate needs
        # ~3.4us of sustained activity to unthrottle from 1.2GHz.
        warm = cpool.tile([128, 2, REP, 128], E16, name="warm_s")
        nc.vector.memset(warm, 0.0)
        wps = psQK.tile([128, REP, 128], F32, name="wps", tag="qk")
        NWARM = 20
        for i in range(NWARM):
            nc.tensor.matmul(wps, warm[:, 0, 0], warm[:, 0],
                             start=(i == 0), stop=(i == NWARM - 1))

        # A(1) interleaved with the first unit's QK/exp pairs (keeps the PE
        # active through the A phase), then A(0), then the 2-deep pipeline.
        u10 = b_alloc(1, 0)
        u10_next = [0]
        for _ in phase_a_gen(1):
            for _ in range(2):
                if u10_next[0] < u10["L"] // 2:
                    b_pair_qk(u10, u10_next[0], fill=True)
                    u10_next[0] += 1
        # A(0) emission interleaved with the rest of u10's (Scalar-paced) QK
        # fill, so its Vector chain runs where Vector is otherwise idle.
        a0 = phase_a_gen(0)
        while u10_next[0] < u10["L"] // 2:
            b_pair_qk(u10, u10_next[0], fill=True)
            u10_next[0] += 1
            next(a0, None)
        for _ in a0:
            pass
        u11 = b_alloc(1, 1)
        u00 = b_alloc(0, 0)
        u01 = b_alloc(0, 1)
        stage(u11, u10)
        stage(u00, u11)
        stage(u01, u00)
        stage(None, u01)
        nc.sync.dma_start(rc_d[:], rc8)

    nc.finalize()
    return nc


# ------------------------- host side -------------------------

def _f16():
    return np.float16


def _host_inputs(core: int, q, k, v):
    c = core
    pad = 7 - c              # packed tile p <-> global tile p - pad
    t0, t1 = c, 8 + c

    kp = np.zeros((S, NHK, HD), np.float32)
    vp = np.zeros((S, NHK, HD), np.float32)
    kp[pad * 128:] = k[:(16 - pad) * 128]
    vp[pad * 128:] = v[:(16 - pad) * 128]

    qq = np.stack([q[t0 * 128:(t0 + 1) * 128], q[t1 * 128:(t1 + 1) * 128]])
    qT = np.ascontiguousarray(qq.transpose(3, 0, 2, 1))   # [128,2,8,128]

    jq = np.arange(QT)
    p = np.arange(128)
    caus = (jq[None, :] >= p[:, None]).astype(np.float32)        # [128,128]
    band = (jq[None, :] < p[:, None]).astype(np.float32)
    caus4 = np.tile(caus, (1, REP))
    winb4 = np.tile(band, (1, REP))
    winm = np.zeros((128, 4, 512), np.float32)
    for i in range(4):
        kt = 3 + i
        if kt - pad < 0:
            continue
        m = band if i == 0 else np.ones((128, 128), np.float32)
        winm[:, i] = np.tile(m, (1, REP))

    # valid packed blocks per (slot, query-row): packed block b is valid iff
    # 4*pad <= b < nvalid_packed(q).  The lower bound is folded into iota64
    # (padded blocks get 1e9 so the on-chip is_ge comparison masks them).
    nvalid = np.zeros((128, 2), np.float32)
    bon = np.zeros((128, 2, NB), np.float32)
    b = np.arange(NB)
    for j, t in ((0, t0), (1, t1)):
        nvalid[:, j] = 4 * pad + (t * 128 + jq + 1) // BLK
        bon[jq, j, 4 * pad] += 1e6
        bon[jq, j, 4 * (t + 7 - c) + jq // BLK] += 1e6
    io64 = np.where(b >= 4 * pad, b, 1e9).astype(np.float32)

    pk = np.arange(S)
    ex01 = ((pk[None, :] // BLK == b[:, None])
            & (b[:, None] >= 4 * pad)).astype(np.float32)

    kcT = kp.reshape(NB, BLK, NHK, HD).mean(1)             # [64,2,128] f32
    vcs = vp.reshape(NB, BLK, NHK, HD).mean(1)

    return {
        "qTp": qT.astype(_f16()),
        "qT32": qT,
        "kTp": np.ascontiguousarray(kp.transpose(2, 1, 0)).astype(_f16()),
        "vbp": np.ascontiguousarray(
            vp.reshape(16, 128, NHK, HD).transpose(1, 2, 0, 3)
        ).astype(_f16()),
        "kcT": np.ascontiguousarray(kcT.transpose(2, 1, 0)),
        "vcs": np.ascontiguousarray(vcs).astype(_f16()),
        "caus4": caus4.reshape(128, REP, 128).astype(_f16()),
        "winm4": winm.reshape(128, 4, REP, 128).astype(_f16()),
        "winb4": winb4.reshape(128, REP, 128).astype(_f16()),
        "nvalid": nvalid,
        "bonus": bon,
        "iota64": np.broadcast_to(io64, (128, NB)).copy(),
        "expand01": ex01.astype(_f16()),
        "identh": np.eye(128, dtype=np.float32).astype(_f16()),
        "ones128": np.ones((128, 128), np.float32).astype(_f16()),
    }


_CACHE = {}


def kernel(q, k, v, g_win, g_cmp, g_slt):
    q = np.asarray(q, np.float32)
    k = np.asarray(k, np.float32)
    v = np.asarray(v, np.float32)
    g_win = np.asarray(g_win, np.float32)
    g_cmp = np.asarray(g_cmp, np.float32)
    g_slt = np.asarray(g_slt, np.float32)

    from concourse.bass_utils import run_bass_kernel_spmd

    if "nc" not in _CACHE:
        _CACHE["nc"] = build_nc()
    nc = _CACHE["nc"]

    in_maps = [_host_inputs(c, q, k, v) for c in range(NCORE)]
    import os
    res = run_bass_kernel_spmd(
        nc, in_maps, core_ids=list(range(NCORE)),
        trace=bool(int(os.environ.get("NSA_TRACE", "0"))),
    )
    _CACHE["last_result"] = res

    out = np.empty((S, NHQ, HD), np.float32)
    for c in range(NCORE):
        r = res.results[c]
        opvw = r["opvw"].astype(np.float32)
        opvs = r["opvs"].astype(np.float32)
        zr = r["zrows"]
        ocp, rc = r["ocp"].astype(np.float32), r["rc"]
        for j, t in ((0, c), (1, 8 + c)):
            qs = slice(t * 128, (t + 1) * 128)
            for g in range(NHK):
                # [128d, 4, 128jq] -> [jq, r, d]
                ow = opvw[j, g].reshape(HD, REP, QT) / zr[j, g, 0].reshape(REP, QT)
                os_ = opvs[j, g].reshape(HD, REP, QT) / zr[j, g, 1].reshape(REP, QT)
                oc = ocp[j, g].reshape(QT, REP, HD) * rc[:, j, g * REP:(g + 1) * REP][:, :, None]
                hs = slice(g * REP, (g + 1) * REP)
                out[qs, hs] = (
                    g_win[qs, hs, None] * ow.transpose(2, 1, 0)
                    + g_slt[qs, hs, None] * os_.transpose(2, 1, 0)
                    + g_cmp[qs, hs, None] * oc
                )
    return out
